# revision 24
# baseline (speedup 1.0000x reference)
"""TRN2 Bass kernel for nn_AttentionBlock (N=4, C=256, L=4096, 4 heads, AGGR=4).

Sharding: 8 cores = (batch n, L-half). Core c handles n=c//2, query positions
l in [half*2048, (half+1)*2048). Each core computes k/v from the full
aggregated sequence of its batch (L2=1024) and produces the full output slice
out[n][:, l_half] -- no cross-core reduction needed.

Host-side layout prep (all exact / input-independent weight algebra):
  - x is handed to each core with its own query half first, then
    DEINTERLEAVED into 4 g-blocks: x'[:, g*1024 + m] = x_half[:, 4m+g], so
    every pooling operand (g fixed, m range) is a PACKED column block.
    Attention is permutation-invariant over keys; the query/output column
    permutation is undone on the host.
  - x' is bf16: pooling TTs then run in the DVE 2x mode (0.52 ns/col).
  - BN folds into Wa (scale) + bias t. Wa folds through Wk/Wv:
    wka = Wk@(s*Wa), wva = Wv@(s*Wa) -- no xa tensor on device.
  - k-path bias (Wk@t + bk) adds a per-query constant to every logit ->
    softmax-invariant -> dropped. v-path bias (Wv@t + bv) is constant over
    keys -> passes through the normalized softmax -> folded into
    bo_eff = Wo@(Wv@t + bv) + bo. Only bq and bo_eff exist on device.
  - avg+max pooling: device computes only pa = (g0+g1)+(g2+g3) and
    pm = max(max(g0,g1),max(g2,g3)) (6 packed-bf16 TTs per chunk); the
    0.25*pa + pm combine is folded into a K=512 stacked weight
    wka4 = [0.25*wka; wka] (likewise wva4), contracted over [pa; pm].

Attention uses the S^T layout (keys on partitions): the softmax denominator
comes free from an appended ones-column in v'^T; normalization is
reciprocal + partition-broadcast + multiply. Matmuls run f32r (full PE rate)
via zero-cost bitcasts except q/k/v projections whose inputs are bf16.
The exp stream on ACT (64 x [128,1024] tiles, ~66.4us) is the pacing
engine; S-matmuls run >=1 m-tile ahead of exp and o-matmuls trail by 3 so
PE never blocks the exp cadence, including across iteration boundaries.
PSUM->SBUF moves for k/v go on the Pool engine; pooling + q-bias +
softmax-normalize + output-bias are DVE; the first q bias and the tail
output biases go on ACT (idle outside the exp stream).
"""

import numpy as np

N, C, L = 4, 256, 4096
HEAD_DIM = 64
H = C // HEAD_DIM          # 4 heads
AGGR = 4
L2 = L // AGGR             # 1024 aggregated positions
LH = L // 2                # 2048 query positions per core
BN_EPS = 1e-5
N_CORES = 8

_CACHE = {}


def _build_program():
    import concourse.bass as bass
    import concourse.bacc as bacc
    import concourse.tile as tile
    from concourse import mybir
    from contextlib import ExitStack

    dt = mybir.dt
    f32 = dt.float32
    f32r = dt.float32r
    bf16 = dt.bfloat16
    AF = mybir.ActivationFunctionType
    Alu = mybir.AluOpType

    nc = bacc.Bacc("TRN2", debug=False, num_devices=N_CORES)

    xf_d = nc.dram_tensor("x_full", [C, L], bf16, kind="ExternalInput")
    # packed weight blobs: wqk = [wqt(2 blk); wka4(4 blk)]
    wqk_d = nc.dram_tensor("wqk", [3 * C, C], bf16, kind="ExternalInput")
    wva_d = nc.dram_tensor("wva4", [2 * C, C], bf16, kind="ExternalInput")
    wot_d = nc.dram_tensor("wot", [C, C], bf16, kind="ExternalInput")
    # host-prelaid per-partition bias: cols (ct0: bq, bo), (ct1: bq, bo)
    bp_d = nc.dram_tensor("biasp", [128, 4], f32, kind="ExternalInput")
    out_d = nc.dram_tensor("out", [C, LH], f32, kind="ExternalOutput")

    with tile.TileContext(nc) as tc, ExitStack() as ctx:
        pp = ctx.enter_context(tc.tile_pool(name="persist", bufs=1))
        at_pool = ctx.enter_context(tc.tile_pool(name="at", bufs=8))
        oa_pool = ctx.enter_context(tc.tile_pool(name="oa", bufs=2))
        outp = ctx.enter_context(tc.tile_pool(name="outp", bufs=3))
        r_pool = ctx.enter_context(tc.tile_pool(name="rp", bufs=2))
        R_pool = ctx.enter_context(tc.tile_pool(name="Rp", bufs=3))

        ps_s = ctx.enter_context(tc.tile_pool(name="ps_s", bufs=3, space="PSUM"))
        ps_o = ctx.enter_context(tc.tile_pool(name="ps_o", bufs=2, space="PSUM"))

        # ---- persistent tiles ----
        xf = [pp.tile([128, L], bf16, name=f"xf{ct}", tag=f"xf{ct}")
              for ct in range(2)]
        q_r = [pp.tile([128, LH], bf16, name=f"qr{ct}", tag=f"qr{ct}")
               for ct in range(2)]
        k_r = [pp.tile([128, L2], bf16, name=f"kr{ct}", tag=f"kr{ct}")
               for ct in range(2)]
        # pooled avg-part / max-part, bf16 (packed for DVE 2x + bf16 matmul)
        pa_r = [pp.tile([128, L2], bf16, name=f"par{ct}", tag=f"par{ct}")
                for ct in range(2)]
        pm_r = [pp.tile([128, L2], bf16, name=f"pmr{ct}", tag=f"pmr{ct}")
                for ct in range(2)]
        # v'^T per m-tile: 4 heads x (64 cols + ones col)
        v_r = [pp.tile([128, 4 * 65], bf16, name=f"vr{mt}", tag=f"vr{mt}")
               for mt in range(8)]
        wqk_t = pp.tile([128, 6 * 256], bf16, name="wqk_t", tag="wqk_t")
        wva_t = pp.tile([128, 4 * 256], bf16, name="wva_t", tag="wva_t")
        wot_t = pp.tile([128, 2 * 256], bf16, name="wot_t", tag="wot_t")
        # cols: (ct0: bq, bo), (ct1: bq, bo)
        bias_t = pp.tile([128, 4], f32, name="bias_t", tag="bias_t")

        def wblk(tile_, blk, off, width=128):
            return tile_[:, blk * 256 + off:blk * 256 + off + width]

        # ---- DMAs ----
        # HWDGE processes setups serially (~630ns each), so: the front half
        # of x' (all 4 g-blocks, m<512 -- covers every q slice and the first
        # pool chunks) + the packed weights go on SP/HWDGE; the back half of
        # x' goes through the gpsimd SWDGE queue, which runs in parallel.
        def x_wave(ct, a, b, eng):
            dst = xf[ct][:].rearrange("p (g m) -> p g m", g=4)[:, :, a:b]
            src = xf_d.ap()[ct * 128:(ct + 1) * 128] \
                .rearrange("p (g m) -> p g m", g=4)[:, :, a:b]
            eng.dma_start(dst, src)

        # transfer order = dependency-deadline order (HBM bandwidth is
        # globally shared, so the byte stream is one serial resource)
        x_wave(0, 0, 512, nc.sync)                     # ct0 front half
        nc.sync.dma_start(xf[1][:, 0:512],             # ct1 q-slice (g0)
                          xf_d.ap()[128:256, 0:512])
        nc.scalar.dma_start(                           # wqt+wka
            wqk_t[:].rearrange("p (k o) -> p k o", k=6),
            wqk_d.ap().rearrange("(k p) o -> p k o", p=128))
        nc.scalar.dma_start(bias_t[:], bp_d.ap())
        # ct1 g1..g3 leading cols, then the rest of the front half
        nc.sync.dma_start(
            xf[1][:, 1024:].rearrange("p (g m) -> p g m", g=3)[:, :, 0:128],
            xf_d.ap()[128:256, 1024:].rearrange(
                "p (g m) -> p g m", g=3)[:, :, 0:128])
        nc.sync.dma_start(
            xf[1][:, 1024:].rearrange("p (g m) -> p g m", g=3)[:, :, 128:512],
            xf_d.ap()[128:256, 1024:].rearrange(
                "p (g m) -> p g m", g=3)[:, :, 128:512])
        nc.scalar.dma_start(
            wva_t[:].rearrange("p (k o) -> p k o", k=4),
            wva_d.ap().rearrange("(k p) o -> p k o", p=128))
        x_wave(0, 512, 1024, nc.sync)                  # ct0 back half
        x_wave(1, 512, 1024, nc.scalar)                # ct1 back half
        nc.scalar.dma_start(
            wot_t[:].rearrange("p (k o) -> p k o", k=2),
            wot_d.ap().rearrange("(k p) o -> p k o", p=128))

        # ---- gpsimd constants; ACT exp-table warm ----
        warm = pp.tile([1, 8], f32, name="warm", tag="warm")
        ones64 = pp.tile([1, 64], bf16, name="ones64", tag="ones64")
        nc.gpsimd.memset(warm[:], 1.0)
        nc.gpsimd.memset(ones64[:], 1.0)
        for mt in range(8):
            nc.gpsimd.memset(
                v_r[mt][:].rearrange("p (h e) -> p h e", e=65)[:, :, 64], 1.0)
        nc.scalar.activation(warm[:], warm[:], AF.Exp, scale=1.0)
        # PE p-state warm-up: a tiny early matmul starts the ramp clock
        # (borrows a ps_o-tagged tile so no extra PSUM tag is allocated)
        ps_w = ps_o.tile([65, 512], f32, name="ps_o", tag="ps_o")
        nc.tensor.matmul(ps_w[0:8, 0:8], warm[0:1, 0:8], warm[0:1, 0:8],
                         start=True, stop=True)

        # ---- pooling: 6 packed bf16 TTs -> pa (sum4), pm (max4) ----
        scrA = pp.tile([128, 512], bf16, name="scrA", tag="scrA")
        scrB = pp.tile([128, 512], bf16, name="scrB", tag="scrB")

        def pool_chunk(ct, p0, pw):
            g = [xf[ct][:, gg * 1024 + p0: gg * 1024 + p0 + pw]
                 for gg in range(4)]
            eng = nc.vector
            eng.tensor_tensor(scrA[:, 0:pw], g[0], g[1], Alu.add)
            eng.tensor_tensor(scrB[:, 0:pw], g[2], g[3], Alu.add)
            eng.tensor_tensor(pa_r[ct][:, p0:p0 + pw], scrA[:, 0:pw],
                              scrB[:, 0:pw], Alu.add)
            eng.tensor_tensor(scrA[:, 0:pw], g[0], g[1], Alu.max)
            eng.tensor_tensor(scrB[:, 0:pw], g[2], g[3], Alu.max)
            eng.tensor_tensor(pm_r[ct][:, p0:p0 + pw], scrA[:, 0:pw],
                              scrB[:, 0:pw], Alu.max)

        # ---- projections ----
        def kv_src(kk):
            # K=512 stacking: k-tiles 0,1 read pa (x0.25 folded in weights),
            # k-tiles 2,3 read pm
            return (pa_r if kk < 2 else pm_r)[kk % 2]

        def k_chunk(m0, mw, copy_eng):
            for ct_out in range(2):
                ps = ps_s.tile([128, 512], f32, name="ps_s", tag="ps_s")
                for kk in range(4):
                    nc.tensor.matmul(
                        ps[:, 0:mw],
                        wblk(wqk_t, 2 + kk, ct_out * 128),
                        kv_src(kk)[:, m0:m0 + mw],
                        start=(kk == 0), stop=(kk == 3))
                copy_eng.tensor_copy(k_r[ct_out][:, m0:m0 + mw], ps[:, 0:mw])

        def q_chunk(lcq, bias_eng, cts=(0, 1)):
            # query part of g-block lcq: x' cols [lcq*1024, lcq*1024+512)
            for ct_out in cts:
                ps = ps_s.tile([128, 512], f32, name="ps_s", tag="ps_s")
                for cch in range(2):
                    nc.tensor.matmul(
                        ps[:], wblk(wqk_t, cch, ct_out * 128),
                        xf[cch][:, lcq * 1024:lcq * 1024 + 512],
                        start=(cch == 0), stop=(cch == 1))
                bq_ap = bias_t[:, 2 * ct_out:2 * ct_out + 1]
                if bias_eng is nc.scalar:
                    nc.scalar.add(q_r[ct_out][:, lcq * 512:(lcq + 1) * 512],
                                  ps[:], bq_ap)
                else:
                    bias_eng.tensor_scalar(
                        q_r[ct_out][:, lcq * 512:(lcq + 1) * 512], ps[:],
                        bq_ap, None, Alu.add)

        def v_block(mt, copy_eng=None):
            # v'^T [m-tile 128, 4 heads x 64] = [pa;pm][:, m-tile]^T @ wva4
            pv = ps_s.tile([128, C], f32, name="ps_s", tag="ps_s")
            for kk in range(4):
                nc.tensor.matmul(
                    pv[:], kv_src(kk)[:, mt * 128:(mt + 1) * 128],
                    wblk(wva_t, kk, 0, 256),
                    start=(kk == 0), stop=(kk == 3))
            vv = v_r[mt][:].rearrange("p (h e) -> p h e", e=65)
            src_ap = pv[:].rearrange("p (h e) -> p h e", e=64)
            if copy_eng is nc.scalar:
                # ACT copy rides the iteration-0 exp-stall gaps
                nc.scalar.copy(vv[:, :, 0:64], src_ap)
            else:
                nc.vector.tensor_copy(vv[:, :, 0:64], src_ap)

        # ---- prefix: minimal chain to the first exp; the rest of the
        # pooling/k/v work is hook-interleaved into iteration (0,0) in
        # deadline order (emission order IS execution order per engine) ----
        pool_chunk(0, 0, 128)
        pool_chunk(1, 0, 128)
        q_chunk(0, nc.scalar)          # bias on ACT: idle before exp stream
        k_chunk(0, 128, nc.vector)

        # ---- attention loop ----
        oa_tiles = {}

        def norm_prev(state, tail=False):
            lc, hp, po = state
            oa = oa_tiles[lc]
            if tail:
                # stage-interleaved so the two h2 chains pipeline DVE/Pool
                r_t = [r_pool.tile([1, 512], f32, name="r", tag="r")
                       for _ in range(2)]
                R_t = [R_pool.tile([64, 512], f32, name="R", tag="R")
                       for _ in range(2)]
                for h2 in range(2):
                    nc.vector.reciprocal(r_t[h2][:], po[h2][64:65, :])
                    nc.gpsimd.partition_broadcast(R_t[h2][:], r_t[h2][:],
                                                  channels=64)
                for h2 in range(2):
                    nc.vector.tensor_tensor(
                        oa[hp][h2 * 64:(h2 + 1) * 64, :], po[h2][0:64, :],
                        R_t[h2][:], Alu.mult)
                return
            for h2 in range(2):
                if True:
                    r_t = r_pool.tile([1, 512], f32, name="r", tag="r")
                    nc.vector.reciprocal(r_t[:], po[h2][64:65, :])
                    R_t = R_pool.tile([64, 512], f32, name="R", tag="R")
                    nc.gpsimd.partition_broadcast(R_t[:], r_t[:], channels=64)
                    nc.vector.tensor_tensor(
                        oa[hp][h2 * 64:(h2 + 1) * 64, :], po[h2][0:64, :],
                        R_t[:], Alu.mult)

        def wo_prev(state, bias_eng=None):
            lc, hp, po = state
            if hp != 1:
                return
            oa = oa_tiles[lc]
            dma_eng = {0: nc.sync, 1: nc.scalar if bias_eng is nc.scalar
                       else nc.sync}
            for ct_out in range(2):
                psW = ps_s.tile([128, 512], f32, name="ps_s", tag="ps_s")
                for cch in range(2):
                    nc.tensor.matmul(
                        psW[:], wblk(wot_t, cch, ct_out * 128),
                        oa[cch][:],
                        start=(cch == 0), stop=(cch == 1))
                out_t = outp.tile([128, 512], f32, name="out", tag="out")
                bo_ap = bias_t[:, 2 * ct_out + 1:2 * ct_out + 2]
                if bias_eng is nc.scalar and ct_out == 0:
                    nc.scalar.add(out_t[:], psW[:], bo_ap)
                else:
                    nc.vector.tensor_scalar(out_t[:], psW[:],
                                            bo_ap, None, Alu.add)
                dma_eng[ct_out].dma_start(
                    out_d.ap()[ct_out * 128:(ct_out + 1) * 128,
                               lc * 512:(lc + 1) * 512], out_t[:])
            del oa_tiles[lc]

        # pending o-matmul FIFO: popped one per (S, exp) step, crossing
        # iteration boundaries so PE never waits on the last exp of an iter
        pending = []

        def emit_iter(lc, hp, prev_state, hooks=None, last=False):
            hooks = hooks or {}
            if hp == 0:
                oa_tiles[lc] = [
                    oa_pool.tile([128, 512], bf16, name=f"oa{ct}",
                                 tag=f"oa{ct}") for ct in range(2)]
            po = [ps_o.tile([65, 512], f32, name="ps_o", tag="ps_o")
                  for _ in range(2)]

            def make_o(mt, at):
                def emit():
                    for h2 in range(2):
                        h = 2 * hp + h2
                        nc.tensor.matmul(
                            po[h2][:], v_r[mt][:, h * 65:h * 65 + 65],
                            at[:, h2 * 512:(h2 + 1) * 512],
                            start=(mt == 0), stop=(mt == 7))
                return emit

            for mt in range(8):
                for th in hooks.get(mt, ()):
                    th()
                ps = ps_s.tile([128, L2], f32, name="ps_s", tag="ps_s")
                for h2 in range(2):
                    nc.tensor.matmul(
                        ps[:, h2 * 512:(h2 + 1) * 512],
                        k_r[hp][h2 * 64:(h2 + 1) * 64,
                                mt * 128:(mt + 1) * 128],
                        q_r[hp][h2 * 64:(h2 + 1) * 64,
                                lc * 512:(lc + 1) * 512],
                        start=True, stop=True)
                at = at_pool.tile([128, 1024], bf16, name="at", tag="at")
                nc.scalar.activation(at[:], ps[:], AF.Exp, scale=0.125)
                pending.append(make_o(mt, at))
                # the previous iteration's last o-matmul pops during step mt1,
                # so its normalize may be emitted no earlier than mt2; the
                # wo waits on the normalize chain (~2us), so it goes at mt6
                if mt == 2 and prev_state is not None:
                    norm_prev(prev_state)
                if mt == 6 and prev_state is not None:
                    wo_prev(prev_state)
                keep = 1 if (last and mt >= 6) else 2
                while len(pending) > keep:
                    pending.pop(0)()
            return (lc, hp, po)

        def mk(f, *a):
            return lambda: f(*a)

        # pooling + k/v chunks hooked into the S stream in DEADLINE order
        # (S(mt) needs k[mt*128..]; o(mt) pops at step mt+2 and needs v(mt))
        hooks00 = {
            0: [mk(pool_chunk, 0, 128, 128), mk(pool_chunk, 1, 128, 128),
                mk(k_chunk, 128, 128, nc.vector)],
            1: [mk(pool_chunk, 0, 256, 256), mk(pool_chunk, 1, 256, 256),
                mk(k_chunk, 256, 256, nc.vector), mk(v_block, 0, nc.scalar)],
            2: [mk(v_block, 1, nc.scalar)],
            3: [mk(pool_chunk, 0, 512, 256), mk(pool_chunk, 1, 512, 256),
                mk(k_chunk, 512, 256, nc.vector), mk(v_block, 2, nc.scalar)],
            4: [mk(v_block, 3, nc.scalar)],
            5: [mk(pool_chunk, 0, 768, 256), mk(pool_chunk, 1, 768, 256),
                mk(k_chunk, 768, 256, nc.vector), mk(v_block, 4)],
            7: [mk(v_block, 5)],
        }
        hooks01 = {
            0: [mk(v_block, 6)],
            1: [mk(v_block, 7)],
            2: [mk(q_chunk, 1, nc.scalar, (0,))],
            3: [mk(q_chunk, 1, nc.scalar, (1,))],
        }

        state = emit_iter(0, 0, None, hooks=hooks00)
        state = emit_iter(0, 1, state, hooks=hooks01)
        state = emit_iter(1, 0, state)
        state = emit_iter(1, 1, state, hooks={
            1: [mk(q_chunk, 2, nc.vector, (0,))],
            2: [mk(q_chunk, 2, nc.vector, (1,))]})
        state = emit_iter(2, 0, state)
        state = emit_iter(2, 1, state, hooks={
            1: [mk(q_chunk, 3, nc.vector, (0,))],
            2: [mk(q_chunk, 3, nc.vector, (1,))]})
        state = emit_iter(3, 0, state)
        state = emit_iter(3, 1, state, last=True)
        while pending:
            pending.pop(0)()

        # ---- tail: column-split pipeline so the two 256-col half-chains
        # (recip -> broadcast -> normalize -> Wo -> bias -> DMA) overlap ----
        lc, hp, po = state
        oa = oa_tiles[lc]
        for half in range(2):
            hs = slice(half * 256, half * 256 + 256)
            for h2 in range(2):
                r_t = r_pool.tile([1, 512], f32, name="r", tag="r")
                nc.vector.reciprocal(r_t[:, hs], po[h2][64:65, hs])
                R_t = R_pool.tile([64, 512], f32, name="R", tag="R")
                nc.gpsimd.partition_broadcast(R_t[:, hs], r_t[:, hs],
                                              channels=64)
                nc.vector.tensor_tensor(
                    oa[hp][h2 * 64:(h2 + 1) * 64, hs], po[h2][0:64, hs],
                    R_t[:, hs], Alu.mult)
            for ct_out in range(2):
                psW = ps_s.tile([128, 512], f32, name="ps_s", tag="ps_s")
                for cch in range(2):
                    nc.tensor.matmul(
                        psW[:, 0:256], wblk(wot_t, cch, ct_out * 128),
                        oa[cch][:, hs],
                        start=(cch == 0), stop=(cch == 1))
                out_t = outp.tile([128, 512], f32, name="out", tag="out")
                bo_ap = bias_t[:, 2 * ct_out + 1:2 * ct_out + 2]
                if ct_out == 0:
                    nc.scalar.add(out_t[:, 0:256], psW[:, 0:256], bo_ap)
                else:
                    nc.vector.tensor_scalar(out_t[:, 0:256], psW[:, 0:256],
                                            bo_ap, None, Alu.add)
                (nc.sync if ct_out == 0 else nc.scalar).dma_start(
                    out_d.ap()[ct_out * 128:(ct_out + 1) * 128,
                               lc * 512 + half * 256:
                               lc * 512 + half * 256 + 256],
                    out_t[:, 0:256])
        del oa_tiles[lc]

    nc.compile()
    return nc


def _get_program():
    if "nc" not in _CACHE:
        _CACHE["nc"] = _build_program()
    return _CACHE["nc"]


def _host_weights(Wq, bq, Wk, Wv, bv, Wo, bo, Wa,
                  g1, b1, m1, v1, g2, b2, m2, v2):
    import ml_dtypes
    # fold both eval-mode BNs into a per-channel affine: xa = s*(Wa@p) + t
    s1 = np.asarray(g1) / np.sqrt(np.asarray(v1) + BN_EPS)
    t1 = np.asarray(b1) - np.asarray(m1) * s1
    s2 = np.asarray(g2) / np.sqrt(np.asarray(v2) + BN_EPS)
    t2 = np.asarray(b2) - np.asarray(m2) * s2
    s = (s1 * s2).astype(np.float32)
    t = (t1 * s2 + t2).astype(np.float32)

    Wk_ = np.asarray(Wk, dtype=np.float32)
    Wv_ = np.asarray(Wv, dtype=np.float32)
    Wo_ = np.asarray(Wo, dtype=np.float32)
    Was = np.asarray(Wa, dtype=np.float32) * s[:, None]
    wka = Wk_ @ Was            # k = wka @ p_combined
    wva = Wv_ @ Was
    # v-path bias (constant over keys) passes through the normalized softmax:
    # fold through Wo into the output bias. k-path bias is softmax-invariant.
    bveff = Wv_ @ t + np.asarray(bv, dtype=np.float32)
    bo_eff = Wo_ @ bveff + np.asarray(bo, dtype=np.float32)

    def stack4(w):
        # [0.25*w.T ; w.T] : K=512 over [pa ; pm]
        wt = w.T.astype(np.float32)
        return np.concatenate([0.25 * wt, wt], axis=0)

    bf = ml_dtypes.bfloat16
    wqt = np.asarray(Wq, dtype=np.float32).T
    return {
        "wqk": np.concatenate([wqt, stack4(wka)], axis=0).astype(bf),
        "wva4": stack4(wva).astype(bf),
        "wot": Wo_.T.copy().astype(bf),
        "biasp": np.stack([np.asarray(bq, dtype=np.float32)[0:128],
                           bo_eff[0:128],
                           np.asarray(bq, dtype=np.float32)[128:256],
                           bo_eff[128:256]], axis=1).astype(np.float32),
    }


def _deint(xs):
    # x'[:, g*1024 + m] = xs[:, 4m+g]
    import ml_dtypes
    return np.ascontiguousarray(
        xs.reshape(C, L // AGGR, AGGR).transpose(0, 2, 1).reshape(C, L)
    ).astype(ml_dtypes.bfloat16)


def _reint(res):
    # out[:, 4j+g] = res[:, g*512+j]
    return np.ascontiguousarray(
        res.reshape(C, 4, LH // 4).transpose(0, 2, 1).reshape(C, LH))


def kernel(x, Wq, bq, Wk, bk, Wv, bv, Wo, bo, Wa,
           g1, b1, m1, v1, g2, b2, m2, v2):
    from concourse import bass_utils

    nc = _get_program()
    x = np.asarray(x, dtype=np.float32)
    shared = _host_weights(Wq, bq, Wk, Wv, bv, Wo, bo, Wa,
                           g1, b1, m1, v1, g2, b2, m2, v2)
    in_maps = []
    for c in range(N_CORES):
        n, half = c // 2, c % 2
        m = dict(shared)
        xs = x[n]
        if half == 1:
            # core's own query half first; key order is irrelevant
            xs = np.concatenate([xs[:, LH:], xs[:, :LH]], axis=1)
        m["x_full"] = _deint(xs)
        in_maps.append(m)

    res = bass_utils.run_bass_kernel_spmd(nc, in_maps,
                                          core_ids=list(range(N_CORES)))
    out = np.empty((N, C, L), np.float32)
    for c in range(N_CORES):
        n, half = c // 2, c % 2
        out[n][:, half * LH:(half + 1) * LH] = _reint(
            np.asarray(res.results[c]["out"]))
    return out
